# revision 2
# baseline (speedup 1.0000x reference)
"""HarmonicOscillator Trainium2 kernel.

Math: out[n,t] = (1/16) * sum_h exp(amps)_up[n,h,t] * sin(2*pi*(h+1)*dt[n,t]),
dt = cumsum(interp(max(f0,20))/48000). Linear interpolation commutes with the
harmonic multiplier, so one per-sample phase suffices; the host computes exact
(fp64) per-segment boundary phases, per-harmonic fractional bases b_h, and
per-(segment,harmonic) amplitude-line coefficients.

Device pipeline, per 128-segment-row tile (4 tiles/core), per h-batch step g
(4 harmonics):

  DVE   P_k = (h+1)*W + (768 + b_h)            k=0,1  tensor_scalar fp32
  ACT   P_k = Identity((h+1)*W + (768 + b_h))  k=2,3  per-partition bias AP
  DVE   F   = P & 0xFFC03FFF                   one op: 768 + frac(P)
  ACT   S   = Sin(2pi*F - 2pi*768.5) -> fp16   = -sin(2pi*frac(P))
  PE    A_cls += diag(c_cls,h) @ S_half        16 fp16 matmuls, fp32 PSUM

then per tile on DVE: res = A0 + J*A1 per segment half (amplitude is linear
in the sample index within each half), and DMA out.

Key tricks:
- frac() via one bitwise AND: P+768 lies in [768, 1024) (fixed exponent 2^9),
  so integer-phase mantissa bits 14..21 are cleared by the mask and the
  fractional phase survives at 2^-14 resolution.
- Sin's table is only valid on [-pi, pi]: the ACT affine maps 768+frac to
  2pi*(frac-0.5), whose sin is -sin(2pi*frac); the sign is absorbed by
  negating the amplitude coefficients on the host.
- The harmonic accumulation runs on the TensorEngine as diagonal-weight
  fp16 matmuls accumulating in PSUM (start at h==0, stop at h==15), with
  each of the 4 accumulators (A0/A1 x segment half) in its own PSUM bank.

Sharding: data-parallel over batch N=16 across 8 cores (2 samples/core).
"""
import sys, math
sys.path.insert(0, '/opt/trn_rl_repo')
import numpy as np

N, NH, LF = 16, 16, 256
SEG = 960
HSEG = 480
SR = 48000.0
LW = LF * SEG              # 245760 waveform samples
NCORES = 8
SPC = N // NCORES          # 2 samples per core
ROWS = SPC * LF            # 512 segment-rows per core
P = 128
NTILES = ROWS // P         # 4
NHB = NH // 4              # 4 h-batch steps of 4 harmonics each
M2 = 768.0
MASK = 0xFFC03FFF
TWO_PI = 2.0 * math.pi

_KERNEL_CACHE = {}


def build_nc(R=1, loop_n=None):
    from concourse import bass, mybir
    from contextlib import ExitStack

    F32 = mybir.dt.float32
    F16 = mybir.dt.float16
    U32 = mybir.dt.uint32
    Alu = mybir.AluOpType
    Act = mybir.ActivationFunctionType

    nc = bass.Bass("TRN2", target_bir_lowering=False, debug=False)

    w_ext = nc.dram_tensor("w", [ROWS, SEG], F32, kind="ExternalInput")
    b_ext = nc.dram_tensor("b", [ROWS, NH], F32, kind="ExternalInput")
    dg_ext = nc.dram_tensor("dg", [ROWS, NH * 4 * P], F16, kind="ExternalInput")
    j_ext = nc.dram_tensor("j", [P, HSEG], F32, kind="ExternalInput")
    o_ext = nc.dram_tensor("o", [ROWS, SEG], F32, kind="ExternalOutput")

    def sb(name, shape, dtype=F32):
        return nc.alloc_sbuf_tensor(name, shape, dtype).ap()

    NF, NS, NP = 3, 4, 2
    W_t = [sb(f"W{x}", [P, SEG]) for x in range(2)]
    B_t = [sb(f"B{x}", [P, NH]) for x in range(2)]
    DG_t = [sb(f"DG{x}", [P, NH * 4 * P], F16) for x in range(2)]
    P_t = [sb(f"P{x}", [P, 4 * SEG]) for x in range(NP)]
    F_t = [sb(f"F{x}", [P, 4 * SEG]) for x in range(NF)]
    S_t = [sb(f"S{x}", [P, 4 * SEG], F16) for x in range(NS)]
    J_t = sb("J", [P, HSEG])
    RES_t = [sb(f"RES{x}", [P, SEG]) for x in range(2)]
    BIAS = sb("BIAS", [P, 1])

    PSB = 512   # PSUM bank stride in fp32; accumulators bank-aligned
    PS = [nc.alloc_psum_tensor(f"PS{x}", [P, 4 * PSB], F32).ap()
          for x in range(2)]

    # exact fp32 bias the ACT affine applies; host compensates its rounding
    bias_val = float(np.float32(-TWO_PI * (M2 + 0.5)))
    nc.gpsimd.memset(BIAS, bias_val)
    nc.all_engine_barrier()

    waited = {}
    T = R * NTILES
    TG = T * NHB

    stack = ExitStack()
    if loop_n is not None:
        stack.enter_context(nc.Fori(0, loop_n))
    with (
        stack,
        nc.semaphore("din_sem") as din_sem,
        nc.semaphore("dout_sem") as dout_sem,
        nc.semaphore("acp_sem") as acp_sem,
        nc.semaphore("vem_sem") as vem_sem,
        nc.semaphore("vec_sem") as vec_sem,
        nc.semaphore("act_sem") as act_sem,
        nc.semaphore("pe_sem") as pe_sem,
    ):
        all_sems = (din_sem, dout_sem, acp_sem, vem_sem, vec_sem, act_sem,
                    pe_sem)
        if loop_n is not None:
            # iteration top: previous block's exit barrier left all engines
            # idle; clear kernel sems, then re-sync before reuse
            for s in all_sems:
                nc.sync.sem_clear(s)
            nc.all_engine_barrier()

        sems = {"din": din_sem, "dout": dout_sem, "acp": acp_sem,
                "vem": vem_sem, "vec": vec_sem, "act": act_sem, "pe": pe_sem}

        def wait(eng, ename, sname, val):
            if val <= 0:
                return
            key = (ename, sname)
            if waited.get(key, -1) >= val:
                return
            waited[key] = val
            eng.wait_ge(sems[sname], val)

        def phase_op(eng, g, k):
            i = g // NHB
            hb = g % NHB
            h = hb * 4 + k
            return eng.tensor_scalar(
                P_t[g % NP][:, k * SEG:(k + 1) * SEG],
                W_t[i % 2], float(h + 1), B_t[i % 2][:, h:h + 1],
                Alu.mult, Alu.add,
            )

        with nc.Block() as block:

            # ---- SP: all DMAs ---------------------------------------------
            @block.sync
            def _(sync):
                sync.dma_start(out=J_t, in_=j_ext.ap()).then_inc(din_sem, 16)
                for i in range(T):
                    s = i % NTILES
                    rs = slice(s * P, (s + 1) * P)
                    if i >= 2:
                        # WAR: W/B read by phases of tile i-2 (DVE implies
                        # via its masks, ACT via acp); DG read by PE of i-2
                        wait(sync, "sp", "vem", 4 * (i - 1))
                        wait(sync, "sp", "acp", 8 * (i - 1))
                        wait(sync, "sp", "pe", 4 * (i - 1))
                    sync.dma_start(out=W_t[i % 2], in_=w_ext.ap()[rs, :]
                                   ).then_inc(din_sem, 16)
                    sync.dma_start(out=B_t[i % 2], in_=b_ext.ap()[rs, :]
                                   ).then_inc(din_sem, 16)
                    sync.dma_start(out=DG_t[i % 2], in_=dg_ext.ap()[rs, :]
                                   ).then_inc(din_sem, 16)
                for i in range(T):
                    s = i % NTILES
                    wait(sync, "sp", "vec", 4 * (i + 1))
                    sync.dma_start(out=o_ext.ap()[s * P:(s + 1) * P, :],
                                   in_=RES_t[i % 2]).then_inc(dout_sem, 16)
                sync.wait_ge(dout_sem, 16 * T)

            # ---- ACT: phases k=2,3 (2 steps ahead) + sins -----------------
            @block.scalar
            def _(scalar):
                def act_phase(g):
                    i = g // NHB
                    hb = g % NHB
                    if g % NHB == 0:
                        wait(scalar, "act", "din", 16 + 48 * i + 32)
                    # WAR: P_t[g%NP] read by mask(g-NP)
                    if g >= NP:
                        wait(scalar, "act", "vem", g - NP + 1)
                    for k in (2, 3):
                        h = hb * 4 + k
                        scalar.activation(
                            P_t[g % NP][:, k * SEG:(k + 1) * SEG], W_t[i % 2],
                            Act.Identity, bias=B_t[i % 2][:, h:h + 1],
                            scale=float(h + 1),
                        ).then_inc(acp_sem)

                for g0 in range(min(NP, TG)):
                    act_phase(g0)
                for g in range(TG):
                    wait(scalar, "act", "vem", g + 1)
                    # WAR: S_t[g%NS] read by PE step g-NS
                    if g >= NS:
                        wait(scalar, "act", "pe", g - NS + 2)
                    scalar.activation(
                        S_t[g % NS], F_t[g % NF], Act.Sin,
                        bias=BIAS[:, 0:1], scale=TWO_PI,
                    ).then_inc(act_sem)
                    if g + NP < TG:
                        act_phase(g + NP)

            # ---- DVE: phases k=0,1 + masks + combines ---------------------
            @block.vector
            def _(vector):
                def combine(ip):
                    wait(vector, "ve", "pe", 4 * (ip + 1))
                    if ip >= 2:
                        wait(vector, "ve", "dout", 16 * (ip - 1))
                    ps = PS[ip % 2]
                    res = RES_t[ip % 2]
                    for half in range(2):
                        sl = slice(half * HSEG, (half + 1) * HSEG)
                        a0 = ps[:, (2 * half) * PSB:(2 * half) * PSB + HSEG]
                        a1 = ps[:, (2 * half + 1) * PSB:
                                (2 * half + 1) * PSB + HSEG]
                        vector.tensor_tensor(res[:, sl], a1, J_t, Alu.mult
                                             ).then_inc(vec_sem)
                        vector.tensor_tensor(res[:, sl], res[:, sl], a0,
                                             Alu.add).then_inc(vec_sem)

                for g in range(TG):
                    i = g // NHB
                    hb = g % NHB
                    if hb == 0:
                        wait(vector, "ve", "din", 16 + 48 * i + 32)
                    for k in (0, 1):
                        phase_op(vector, g, k)
                    wait(vector, "ve", "acp", 2 * (g + 1))
                    # WAR: F_t[g%NF] read by sin(g-NF)
                    if g >= NF:
                        wait(vector, "ve", "act", g - NF + 1)
                    vector.tensor_scalar(
                        F_t[g % NF].bitcast(U32), P_t[g % NP].bitcast(U32),
                        MASK, None, Alu.bitwise_and,
                    ).then_inc(vem_sem)
                    if hb == 1 and i >= 1:
                        combine(i - 1)
                combine(T - 1)

            # ---- PE: diag matmuls, one sem inc per step -------------------
            @block.tensor
            def _(tensor):
                for g in range(TG):
                    i = g // NHB
                    hb = g % NHB
                    if hb == 0:
                        wait(tensor, "pe", "din", 16 + 48 * (i + 1))
                        if i >= 2:
                            # WAR: PSUM group reused from tile i-2
                            wait(tensor, "pe", "vec", 4 * (i - 1))
                    wait(tensor, "pe", "act", g + 1)
                    mm = None
                    for k in range(4):
                        h = hb * 4 + k
                        for cls in range(4):
                            half = cls // 2
                            rhs = S_t[g % NS][:, k * SEG + half * HSEG:
                                              k * SEG + (half + 1) * HSEG]
                            lhsT = DG_t[i % 2][:, (h * 4 + cls) * P:
                                               (h * 4 + cls + 1) * P]
                            mm = tensor.matmul(
                                PS[i % 2][:, cls * PSB:cls * PSB + HSEG],
                                lhsT, rhs,
                                start=(h == 0), stop=(h == NH - 1),
                            )
                    mm.then_inc(pe_sem)

    return nc


def _host_precompute(amps, f0):
    """fp64 host side; returns W rows, (768+base) rows, packed diag weights."""
    f0c = np.maximum(f0[:, 0, :].astype(np.float64), 20.0)       # [N, LF]
    t = np.arange(LW, dtype=np.float64)
    pos = np.clip((t + 0.5) / SEG - 0.5, 0.0, LF - 1)
    i0 = np.floor(pos).astype(np.int64)
    i1 = np.minimum(i0 + 1, LF - 1)
    wfrac = pos - i0
    f0_up = f0c[:, i0] * (1.0 - wfrac) + f0c[:, i1] * wfrac       # [N, LW]
    dt = np.cumsum(f0_up / SR, axis=1)                            # inclusive
    bound = np.concatenate(
        [np.zeros((N, 1)), dt[:, SEG - 1::SEG][:, :-1]], axis=1)  # [N, LF]
    W = (dt.reshape(N, LF, SEG) - bound[:, :, None]).astype(np.float32)

    hmul = np.arange(1, NH + 1, dtype=np.float64)                 # [NH]
    base = np.mod(hmul[None, :, None] * bound[:, None, :], 1.0)   # [N,NH,LF]
    # compensate ACT affine rounding: arg = fl(2pi)*F + fl(-2pi*768.5)
    fl2pi = np.float64(np.float32(TWO_PI))
    flbias = np.float64(np.float32(-TWO_PI * (M2 + 0.5)))
    delta = fl2pi * (M2 + 0.5) + flbias        # rad, constant offset
    base = np.mod(base - delta / TWO_PI, 1.0)
    B = (M2 + base).astype(np.float32)                            # [N,NH,LF]

    a = np.exp(amps.astype(np.float64)) / NH                      # [N,NH,LF]
    am = np.concatenate([a[:, :, 0:1], a[:, :, :-1]], axis=2)     # a[s-1]
    d = a - am
    c0a = am + d * (480.5 / SEG)
    c1a = d / SEG
    an = np.concatenate([a[:, :, 1:], a[:, :, -1:]], axis=2)      # a[s+1]
    e = an - a
    # second half uses local j' = j-480: fold 480*c1b into c0b
    c0b = a - e * (479.5 / SEG) + HSEG * (e / SEG)
    c1b = e / SEG
    # negate everything: device computes -sin
    cls = np.stack([-c0a, -c1a, -c0b, -c1b], axis=0)              # [4,N,NH,LF]

    def rows(x):   # [N,NH,LF] -> [N*LF, NH] with row index (n, s)
        return x.transpose(0, 2, 1).reshape(N * LF, NH)

    Wr = W.reshape(N * LF, SEG)                                   # row=(n,s)
    Br = rows(B)                                                  # [N*LF, NH]

    # diag pack: dg[r, (h*4+cls)*P + (r%P)] = cls[c, n, h, s]
    clsr = np.stack([rows(cls[c]) for c in range(4)], axis=2)     # [R,NH,4]
    dg = np.zeros((N * LF, NH * 4 * P), np.float16)
    ridx = np.arange(N * LF)
    col0 = (np.arange(NH * 4) * P)[None, :] + (ridx % P)[:, None]  # [R, 64]
    np.put_along_axis(dg, col0,
                      clsr.reshape(N * LF, NH * 4).astype(np.float16), axis=1)
    return Wr, Br, dg


def make_in_maps(amps, f0):
    Wr, Br, dg = _host_precompute(amps, f0)
    J = np.broadcast_to(np.arange(HSEG, dtype=np.float32), (P, HSEG)).copy()
    in_maps = []
    for c in range(NCORES):
        r0 = c * ROWS
        in_maps.append({
            "w": np.ascontiguousarray(Wr[r0:r0 + ROWS]),
            "b": np.ascontiguousarray(Br[r0:r0 + ROWS]),
            "dg": np.ascontiguousarray(dg[r0:r0 + ROWS]),
            "j": J,
        })
    return in_maps


def kernel(amps, f0):
    from concourse.bass_utils import run_bass_kernel_spmd

    if "nc" not in _KERNEL_CACHE:
        _KERNEL_CACHE["nc"] = build_nc(1)
    nc = _KERNEL_CACHE["nc"]

    in_maps = make_in_maps(amps, f0)
    res = run_bass_kernel_spmd(nc, in_maps, list(range(NCORES)))
    out = np.concatenate(
        [res.results[c]["o"].reshape(SPC, 1, LW) for c in range(NCORES)],
        axis=0)
    return out.astype(np.float32)
